# revision 27
# baseline (speedup 1.0000x reference)
"""L2-bounded LTI cell (SSM scan) as a radix-4 tap conv + one doubling
stage on TRN2.

Math: per batch b the reference computes
    x_{t+1} = A x_t + B u_t          (col-vector convention)
    y_t     = C x_t + D u_t
with x_seq[t] = x_t (pre-update), y_seq[t] = y_t, so

    x_t = sum_{m=0}^{t-1} A^m B u_{t-1-m} + A^t x0.

||A^8||_2 ~ 4.5e-2 and the tail decays fast, so truncating at 8 taps
leaves ~2.4e-4 relative error on x (measured in fp64) — far below the
2e-2 gate. The 8-tap causal conv is evaluated per 512-column tile as

    w4 = (B + ABz + A^2Bz^2 + A^3Bz^3) u   -> 4 bf16 matmuls on u
    x  = w4 + A^4 * (w4 shifted by 4)      -> 1 bf16 "doubling" matmul

all accumulated on one PSUM bank (the "+ w4" term is the partial sum
already sitting in PSUM), plus y = D u + C x (bf16 D-term, fp32r
C-term): 7 matmuls/tile vs 28 for the direct tap conv baseline.

Precision (validated in host simulation and on HW): head taps / relay /
D-term in single-pass bf16 contribute errors that are NOT amplified
(they enter through small-gain paths), giving x ~ 3.8e-3. The C-term's
x input is the one place bf16 is not enough (bf16(x) alone costs 4e-2
on y), so x enters y through a rounded fp32r copy and C stays fp32r:
y ~ 1.2e-2 (gate 2e-2). Outputs are stored bf16 (halves output DMA)
and upcast on host.

Engine budget per 512-col tile and core (measured): 7 matmuls ~ 1.9us
on PE (the only engine near saturation); the 3 PSUM->SBUF copies per
batch (w4 bf16, x32 fp32r, y bf16) alternate between DVE and Scalar
(PSUM reads run at ~96 G elem/s, ~0.7us per copy); u loads + y stores
issue on the Sync HWDGE queue, x stores are gpsimd software-DGE
casting DMAs straight from the fp32r x32 tile (SBUF->DRAM bf16),
keeping all DMA off the copy engines.

Sharding: batch 32 -> 4 per core, 8 cores, SPMD, no collectives.
Layout: (d=128 partitions) x (time free dim); host pre-pads/transposes
u to bf16, post-transposes y/x. All 4 batches' buffers are SBUF
resident for the full T=4096 (bf16 makes this fit), and emission
interleaves the 4 batches at matmul granularity so each batch's
matmul -> PSUM-copy -> matmul chain latency hides behind the other
three batches' matmuls. The tiny x0 A^t boundary term is added on host.
"""

from functools import lru_cache

import numpy as np

B_FULL, T, D = 32, 4096, 128
N_CORES = 8
B_LOCAL = B_FULL // N_CORES  # 4

PAD = 8  # left zero-pad of u / w4 (max shift: 4 head taps + relay 4)
M_X0 = 64  # host-side x0-term horizon; ||A^64|| ~ 0
NT = 512  # matmul free dim (one fp32 PSUM bank)
NTILES = T // NT
OUT_CHUNK = 512  # output DMA granularity (cols)

_last_result = None  # BassKernelResults of the most recent run (for test.py)


def _host_matrices(S, K_raw):
    """Mirror reference._ssm_matrices bit-for-bit: fp32 jax on CPU."""
    import jax
    import jax.numpy as jnp

    cpu = jax.devices("cpu")[0]
    with jax.default_device(cpu):
        d_x = S.shape[0]
        sigma = jnp.maximum(jnp.linalg.norm(jnp.asarray(K_raw), ord=2), 1e-5)
        K = jnp.asarray(K_raw) / (sigma + 0.002)
        K11 = K[:d_x, :d_x]
        K12 = K[:d_x, d_x:]
        K21 = K[d_x:, :d_x]
        K22 = K[d_x:, d_x:]
        Sinv = jnp.linalg.inv(jnp.asarray(S))
        A = Sinv @ K11 @ jnp.asarray(S)
        Bm = Sinv @ K12  # GAMMA = 1.0
        C = K21 @ jnp.asarray(S)
        Dm = K22
        return (np.asarray(A), np.asarray(Bm), np.asarray(C), np.asarray(Dm))


@lru_cache(maxsize=2)
def _build():
    import concourse.mybir as mybir
    import concourse.tile as tile
    from concourse import bacc

    F32 = mybir.dt.float32
    F32R = mybir.dt.float32r
    BF16 = mybir.dt.bfloat16
    UW = T + PAD

    nc = bacc.Bacc("TRN2", target_bir_lowering=False, num_devices=N_CORES)
    u_d = nc.dram_tensor("u", [B_LOCAL, D, UW], BF16, kind="ExternalInput")
    gw_d = nc.dram_tensor("gw", [D, 5, D], BF16, kind="ExternalInput")
    cdb_d = nc.dram_tensor("cdb", [D, D], BF16, kind="ExternalInput")
    cdc_d = nc.dram_tensor("cdc", [D, D], F32R, kind="ExternalInput")
    y_d = nc.dram_tensor("y", [B_LOCAL, D, T], BF16, kind="ExternalOutput")
    x_d = nc.dram_tensor("x", [B_LOCAL, D, T], BF16, kind="ExternalOutput")

    with tile.TileContext(nc) as tc:
        with (
            tc.tile_pool(name="const", bufs=1) as const,
            tc.tile_pool(name="upool", bufs=1) as upool,
            tc.tile_pool(name="wpool", bufs=1) as wpool,
            tc.tile_pool(name="x32p", bufs=2) as x32pool,
            tc.tile_pool(name="yacc", bufs=1) as yaccpool,
            tc.tile_pool(name="px", bufs=1, space="PSUM") as px_pool,
            tc.tile_pool(name="py", bufs=1, space="PSUM") as py_pool,
        ):
            gw_sb = const.tile([D, 5, D], BF16)
            nc.sync.dma_start(gw_sb[:], gw_d[:])

            u_t, w4, yacc, xtail = [], [], [], []
            for b in range(B_LOCAL):
                ut = upool.tile([D, UW], BF16, name=f"u{b}", tag=f"u{b}")
                u_t.append(ut)
            # u loads: 4 chunks per batch, emitted round-robin across
            # batches so no batch's first tiles wait behind another
            # batch's bulk transfer (the PE queue is in-order). The
            # first-tile chunks lead both HWDGE queues, right after gw
            # (cdb/cdc are not needed until the first y matmuls).
            bounds = [0, PAD + NT, PAD + NT + 1192, PAD + NT + 2384, UW]
            lo, hi = bounds[0], bounds[1]
            for b in range(B_LOCAL):
                eng = nc.scalar if b % 2 else nc.sync
                eng.dma_start(u_t[b][:, lo:hi], u_d[b][:, lo:hi])
            cdb_sb = const.tile([D, D], BF16)
            nc.scalar.dma_start(cdb_sb[:], cdb_d[:])
            cdc_sb = const.tile([D, D], F32R)
            nc.scalar.dma_start(cdc_sb[:], cdc_d[:])
            for ci in range(1, 4):
                lo, hi = bounds[ci], bounds[ci + 1]
                for b in range(B_LOCAL):
                    nc.sync.dma_start(u_t[b][:, lo:hi], u_d[b][:, lo:hi])
            for b in range(B_LOCAL):
                wt = wpool.tile([D, UW], BF16, name=f"w{b}", tag=f"w{b}")
                nc.gpsimd.memset(wt[:, :PAD], 0.0)
                w4.append(wt)
                yacc.append(
                    yaccpool.tile([D, T], BF16, name=f"ya{b}", tag=f"ya{b}")
                )
                xtail.append(
                    yaccpool.tile(
                        [D, 2 * NT], BF16, name=f"xt{b}", tag=f"xt{b}"
                    )
                )

            for j in range(NTILES):
                o = j * NT
                px = [None] * B_LOCAL
                py = [None] * B_LOCAL
                x32 = [None] * B_LOCAL
                # head taps: px = sum_m (A^m B) u_{t-1-m}, m = 0..3
                for m in range(4):
                    for b in range(B_LOCAL):
                        if m == 0:
                            px[b] = px_pool.tile(
                                [D, NT], F32, name=f"px{b}", tag=f"px{b}"
                            )
                        nc.tensor.matmul(
                            px[b][:],
                            gw_sb[:, m, :],
                            u_t[b][:, o + PAD - 1 - m : o + PAD - 1 - m + NT],
                            start=(m == 0),
                            stop=False,
                        )
                for b in range(B_LOCAL):
                    eng = nc.vector.tensor_copy if b % 2 == 0 else nc.scalar.copy
                    eng(w4[b][:, o + PAD : o + PAD + NT], px[b][:])
                # relay: px += A^4 * (w4 shifted by 4)  -> x (8 taps)
                for b in range(B_LOCAL):
                    nc.tensor.matmul(
                        px[b][:],
                        gw_sb[:, 4, :],
                        w4[b][:, o + PAD - 4 : o + PAD - 4 + NT],
                        start=False,
                        stop=True,
                    )
                for b in range(B_LOCAL):
                    x32[b] = x32pool.tile(
                        [D, NT], F32R, name=f"x32{b}", tag=f"x32{b}"
                    )
                    eng = nc.scalar.copy if b % 2 == 0 else nc.vector.tensor_copy
                    eng(x32[b][:], px[b][:])
                if j < NTILES - 2:
                    for b in range(B_LOCAL):
                        nc.gpsimd.dma_start(x_d[b][:, o : o + NT], x32[b][:])
                else:
                    ot = (j - (NTILES - 2)) * NT
                    for b in range(B_LOCAL):
                        eng = (
                            nc.vector.tensor_copy
                            if b % 2 == 0
                            else nc.scalar.copy
                        )
                        eng(xtail[b][:, ot : ot + NT], px[b][:])
                    for b in range(B_LOCAL):
                        nc.sync.dma_start(
                            x_d[b][:, o : o + NT], xtail[b][:, ot : ot + NT]
                        )
                # y = D u (bf16) + C x (fp32r)
                for b in range(B_LOCAL):
                    py[b] = py_pool.tile(
                        [D, NT], F32, name=f"py{b}", tag=f"py{b}"
                    )
                    nc.tensor.matmul(
                        py[b][:],
                        cdb_sb[:],
                        u_t[b][:, o + PAD : o + PAD + NT],
                        start=True,
                        stop=False,
                    )
                for b in range(B_LOCAL):
                    nc.tensor.matmul(
                        py[b][:], cdc_sb[:], x32[b][:], start=False, stop=True
                    )
                for b in range(B_LOCAL):
                    eng = nc.scalar.copy if b % 2 == 0 else nc.vector.tensor_copy
                    eng(yacc[b][:, o : o + NT], py[b][:])

                if (o + NT) % OUT_CHUNK == 0:
                    lo = o + NT - OUT_CHUNK
                    for b in range(B_LOCAL):
                        nc.sync.dma_start(
                            y_d[b][:, lo : o + NT], yacc[b][:, lo : o + NT]
                        )
    nc.compile()
    return nc


def _pack_inputs(u, S, K_raw):
    import ml_dtypes

    bf = ml_dtypes.bfloat16
    A, Bm, C, Dm = _host_matrices(S, K_raw)
    A64 = A.astype(np.float64)
    B64 = Bm.astype(np.float64)

    # gw slots 0..3: (A^m B).T head taps; slot 4: (A^4).T relay.
    mats = []
    Am = np.eye(D)
    for m in range(4):
        mats.append((Am @ B64).T)
        Am = A64 @ Am
    mats.append(Am.T)  # (A^4).T relay
    gw_host = np.ascontiguousarray(
        np.stack(mats, axis=1).astype(np.float32)
    ).astype(bf)
    cdb_host = Dm.T.astype(np.float32).astype(bf)
    cdc_host = np.ascontiguousarray(C.T.astype(np.float32))

    in_maps = []
    for c in range(N_CORES):
        up = np.zeros((B_LOCAL, D, PAD + T), dtype=bf)
        for b in range(B_LOCAL):
            up[b, :, PAD:] = u[c * B_LOCAL + b].T.astype(bf)
        in_maps.append(
            {"u": up, "gw": gw_host, "cdb": cdb_host, "cdc": cdc_host}
        )
    return in_maps, A, C


def kernel(u, x0, S, K_raw):
    global _last_result
    from concourse.bass_utils import run_bass_kernel_spmd

    u = np.asarray(u, dtype=np.float32)
    x0 = np.asarray(x0, dtype=np.float32)
    S = np.asarray(S, dtype=np.float32)
    K_raw = np.asarray(K_raw, dtype=np.float32)

    in_maps, A, C = _pack_inputs(u, S, K_raw)
    nc = _build()
    res = run_bass_kernel_spmd(nc, in_maps, core_ids=list(range(N_CORES)))
    _last_result = res

    y_seq = np.empty((B_FULL, T, D), dtype=np.float32)
    x_seq = np.empty((B_FULL, T, D), dtype=np.float32)
    for c in range(N_CORES):
        ry, rx = res.results[c]["y"], res.results[c]["x"]
        for b in range(B_LOCAL):
            y_seq[c * B_LOCAL + b] = ry[b].T.astype(np.float32)
            x_seq[c * B_LOCAL + b] = rx[b].T.astype(np.float32)

    # x0 boundary term: x_t += A^t x0, y_t += C A^t x0, t < M_X0.
    At = A.T.astype(np.float64)
    Ct64 = C.T.astype(np.float64)
    xc = x0.astype(np.float64)
    for t in range(M_X0):
        x_seq[:, t, :] += xc.astype(np.float32)
        y_seq[:, t, :] += (xc @ Ct64).astype(np.float32)
        xc = xc @ At
    return (y_seq, x_seq)


# revision 28
# speedup vs baseline: 1.0015x; 1.0015x over previous
"""L2-bounded LTI cell (SSM scan) as a radix-4 tap conv + one doubling
stage on TRN2.

Math: per batch b the reference computes
    x_{t+1} = A x_t + B u_t          (col-vector convention)
    y_t     = C x_t + D u_t
with x_seq[t] = x_t (pre-update), y_seq[t] = y_t, so

    x_t = sum_{m=0}^{t-1} A^m B u_{t-1-m} + A^t x0.

||A^8||_2 ~ 4.5e-2 and the tail decays fast, so truncating at 8 taps
leaves ~2.4e-4 relative error on x (measured in fp64) — far below the
2e-2 gate. The 8-tap causal conv is evaluated per 512-column tile as

    w4 = (B + ABz + A^2Bz^2 + A^3Bz^3) u   -> 4 bf16 matmuls on u
    x  = w4 + A^4 * (w4 shifted by 4)      -> 1 bf16 "doubling" matmul

all accumulated on one PSUM bank (the "+ w4" term is the partial sum
already sitting in PSUM), plus y = D u + C x (bf16 D-term, fp32r
C-term): 7 matmuls/tile vs 28 for the direct tap conv baseline.

Precision (validated in host simulation and on HW): head taps / relay /
D-term in single-pass bf16 contribute errors that are NOT amplified
(they enter through small-gain paths), giving x ~ 3.8e-3. The C-term's
x input is the one place bf16 is not enough (bf16(x) alone costs 4e-2
on y), so x enters y through a rounded fp32r copy and C stays fp32r:
y ~ 1.2e-2 (gate 2e-2). Outputs are stored bf16 (halves output DMA)
and upcast on host.

Engine budget per 512-col tile and core (measured): 7 matmuls ~ 1.9us
on PE (the only engine near saturation); the 3 PSUM->SBUF copies per
batch (w4 bf16, x32 fp32r, y bf16) alternate between DVE and Scalar
(PSUM reads run at ~96 G elem/s, ~0.7us per copy); u loads + y stores
issue on the Sync HWDGE queue, x stores are gpsimd software-DGE
casting DMAs straight from the fp32r x32 tile (SBUF->DRAM bf16),
keeping all DMA off the copy engines.

Sharding: batch 32 -> 4 per core, 8 cores, SPMD, no collectives.
Layout: (d=128 partitions) x (time free dim); host pre-pads/transposes
u to bf16, post-transposes y/x. All 4 batches' buffers are SBUF
resident for the full T=4096 (bf16 makes this fit), and emission
interleaves the 4 batches at matmul granularity so each batch's
matmul -> PSUM-copy -> matmul chain latency hides behind the other
three batches' matmuls. The tiny x0 A^t boundary term is added on host.
"""

from functools import lru_cache

import numpy as np

B_FULL, T, D = 32, 4096, 128
N_CORES = 8
B_LOCAL = B_FULL // N_CORES  # 4

PAD = 8  # left zero-pad of u / w4 (max shift: 4 head taps + relay 4)
M_X0 = 64  # host-side x0-term horizon; ||A^64|| ~ 0
NT = 512  # matmul free dim (one fp32 PSUM bank)
NTILES = T // NT
OUT_CHUNK = 512  # output DMA granularity (cols)

_last_result = None  # BassKernelResults of the most recent run (for test.py)


def _host_matrices(S, K_raw):
    """Mirror reference._ssm_matrices bit-for-bit: fp32 jax on CPU."""
    import jax
    import jax.numpy as jnp

    cpu = jax.devices("cpu")[0]
    with jax.default_device(cpu):
        d_x = S.shape[0]
        sigma = jnp.maximum(jnp.linalg.norm(jnp.asarray(K_raw), ord=2), 1e-5)
        K = jnp.asarray(K_raw) / (sigma + 0.002)
        K11 = K[:d_x, :d_x]
        K12 = K[:d_x, d_x:]
        K21 = K[d_x:, :d_x]
        K22 = K[d_x:, d_x:]
        Sinv = jnp.linalg.inv(jnp.asarray(S))
        A = Sinv @ K11 @ jnp.asarray(S)
        Bm = Sinv @ K12  # GAMMA = 1.0
        C = K21 @ jnp.asarray(S)
        Dm = K22
        return (np.asarray(A), np.asarray(Bm), np.asarray(C), np.asarray(Dm))


@lru_cache(maxsize=2)
def _build():
    import concourse.mybir as mybir
    import concourse.tile as tile
    from concourse import bacc

    F32 = mybir.dt.float32
    F32R = mybir.dt.float32r
    BF16 = mybir.dt.bfloat16
    UW = T + PAD

    nc = bacc.Bacc("TRN2", target_bir_lowering=False, num_devices=N_CORES)
    u_d = nc.dram_tensor("u", [B_LOCAL, D, UW], BF16, kind="ExternalInput")
    gw_d = nc.dram_tensor("gw", [D, 5, D], BF16, kind="ExternalInput")
    cdb_d = nc.dram_tensor("cdb", [D, D], BF16, kind="ExternalInput")
    cdc_d = nc.dram_tensor("cdc", [D, D], F32R, kind="ExternalInput")
    y_d = nc.dram_tensor("y", [B_LOCAL, D, T], BF16, kind="ExternalOutput")
    x_d = nc.dram_tensor("x", [B_LOCAL, D, T], BF16, kind="ExternalOutput")

    with tile.TileContext(nc) as tc:
        with (
            tc.tile_pool(name="const", bufs=1) as const,
            tc.tile_pool(name="upool", bufs=1) as upool,
            tc.tile_pool(name="wpool", bufs=1) as wpool,
            tc.tile_pool(name="x32p", bufs=2) as x32pool,
            tc.tile_pool(name="yacc", bufs=1) as yaccpool,
            tc.tile_pool(name="px", bufs=1, space="PSUM") as px_pool,
            tc.tile_pool(name="py", bufs=1, space="PSUM") as py_pool,
        ):
            gw_sb = const.tile([D, 5, D], BF16)
            nc.sync.dma_start(gw_sb[:], gw_d[:])
            cdb_sb = const.tile([D, D], BF16)
            nc.sync.dma_start(cdb_sb[:], cdb_d[:])
            cdc_sb = const.tile([D, D], F32R)
            nc.sync.dma_start(cdc_sb[:], cdc_d[:])

            u_t, w4, yacc, xtail = [], [], [], []
            for b in range(B_LOCAL):
                ut = upool.tile([D, UW], BF16, name=f"u{b}", tag=f"u{b}")
                u_t.append(ut)
            # u loads: 4 chunks per batch, emitted round-robin across
            # batches so no batch's first tiles wait behind another
            # batch's bulk transfer (the PE queue is in-order).
            bounds = [0, PAD + NT, PAD + NT + 1192, PAD + NT + 2384, UW]
            for ci in range(4):
                lo, hi = bounds[ci], bounds[ci + 1]
                for b in range(B_LOCAL):
                    eng = nc.scalar if (ci == 0 and b % 2) else nc.sync
                    eng.dma_start(u_t[b][:, lo:hi], u_d[b][:, lo:hi])
            for b in range(B_LOCAL):
                wt = wpool.tile([D, UW], BF16, name=f"w{b}", tag=f"w{b}")
                nc.gpsimd.memset(wt[:, :PAD], 0.0)
                w4.append(wt)
                yacc.append(
                    yaccpool.tile([D, T], BF16, name=f"ya{b}", tag=f"ya{b}")
                )
                xtail.append(
                    yaccpool.tile(
                        [D, 2 * NT], BF16, name=f"xt{b}", tag=f"xt{b}"
                    )
                )

            for j in range(NTILES):
                o = j * NT
                px = [None] * B_LOCAL
                py = [None] * B_LOCAL
                x32 = [None] * B_LOCAL
                # head taps: px = sum_m (A^m B) u_{t-1-m}, m = 0..3
                for m in range(4):
                    for b in range(B_LOCAL):
                        if m == 0:
                            px[b] = px_pool.tile(
                                [D, NT], F32, name=f"px{b}", tag=f"px{b}"
                            )
                        nc.tensor.matmul(
                            px[b][:],
                            gw_sb[:, m, :],
                            u_t[b][:, o + PAD - 1 - m : o + PAD - 1 - m + NT],
                            start=(m == 0),
                            stop=False,
                        )
                for b in range(B_LOCAL):
                    eng = nc.vector.tensor_copy if b % 2 == 0 else nc.scalar.copy
                    eng(w4[b][:, o + PAD : o + PAD + NT], px[b][:])
                # relay: px += A^4 * (w4 shifted by 4)  -> x (8 taps)
                for b in range(B_LOCAL):
                    nc.tensor.matmul(
                        px[b][:],
                        gw_sb[:, 4, :],
                        w4[b][:, o + PAD - 4 : o + PAD - 4 + NT],
                        start=False,
                        stop=True,
                    )
                for b in range(B_LOCAL):
                    x32[b] = x32pool.tile(
                        [D, NT], F32R, name=f"x32{b}", tag=f"x32{b}"
                    )
                    eng = nc.scalar.copy if b % 2 == 0 else nc.vector.tensor_copy
                    eng(x32[b][:], px[b][:])
                if j < NTILES - 2:
                    for b in range(B_LOCAL):
                        nc.gpsimd.dma_start(x_d[b][:, o : o + NT], x32[b][:])
                else:
                    ot = (j - (NTILES - 2)) * NT
                    for b in range(B_LOCAL):
                        eng = (
                            nc.vector.tensor_copy
                            if b % 2 == 0
                            else nc.scalar.copy
                        )
                        eng(xtail[b][:, ot : ot + NT], px[b][:])
                    for b in range(B_LOCAL):
                        nc.sync.dma_start(
                            x_d[b][:, o : o + NT], xtail[b][:, ot : ot + NT]
                        )
                # y = D u (bf16) + C x (fp32r)
                for b in range(B_LOCAL):
                    py[b] = py_pool.tile(
                        [D, NT], F32, name=f"py{b}", tag=f"py{b}"
                    )
                    nc.tensor.matmul(
                        py[b][:],
                        cdb_sb[:],
                        u_t[b][:, o + PAD : o + PAD + NT],
                        start=True,
                        stop=False,
                    )
                for b in range(B_LOCAL):
                    nc.tensor.matmul(
                        py[b][:], cdc_sb[:], x32[b][:], start=False, stop=True
                    )
                for b in range(B_LOCAL):
                    eng = nc.scalar.copy if b % 2 == 0 else nc.vector.tensor_copy
                    eng(yacc[b][:, o : o + NT], py[b][:])

                if (o + NT) % OUT_CHUNK == 0:
                    lo = o + NT - OUT_CHUNK
                    for b in range(B_LOCAL):
                        nc.sync.dma_start(
                            y_d[b][:, lo : o + NT], yacc[b][:, lo : o + NT]
                        )
    nc.compile()
    return nc


def _pack_inputs(u, S, K_raw):
    import ml_dtypes

    bf = ml_dtypes.bfloat16
    A, Bm, C, Dm = _host_matrices(S, K_raw)
    A64 = A.astype(np.float64)
    B64 = Bm.astype(np.float64)

    # gw slots 0..3: (A^m B).T head taps; slot 4: (A^4).T relay.
    mats = []
    Am = np.eye(D)
    for m in range(4):
        mats.append((Am @ B64).T)
        Am = A64 @ Am
    mats.append(Am.T)  # (A^4).T relay
    gw_host = np.ascontiguousarray(
        np.stack(mats, axis=1).astype(np.float32)
    ).astype(bf)
    cdb_host = Dm.T.astype(np.float32).astype(bf)
    cdc_host = np.ascontiguousarray(C.T.astype(np.float32))

    in_maps = []
    for c in range(N_CORES):
        up = np.zeros((B_LOCAL, D, PAD + T), dtype=bf)
        for b in range(B_LOCAL):
            up[b, :, PAD:] = u[c * B_LOCAL + b].T.astype(bf)
        in_maps.append(
            {"u": up, "gw": gw_host, "cdb": cdb_host, "cdc": cdc_host}
        )
    return in_maps, A, C


def kernel(u, x0, S, K_raw):
    global _last_result
    from concourse.bass_utils import run_bass_kernel_spmd

    u = np.asarray(u, dtype=np.float32)
    x0 = np.asarray(x0, dtype=np.float32)
    S = np.asarray(S, dtype=np.float32)
    K_raw = np.asarray(K_raw, dtype=np.float32)

    in_maps, A, C = _pack_inputs(u, S, K_raw)
    nc = _build()
    res = run_bass_kernel_spmd(nc, in_maps, core_ids=list(range(N_CORES)))
    _last_result = res

    y_seq = np.empty((B_FULL, T, D), dtype=np.float32)
    x_seq = np.empty((B_FULL, T, D), dtype=np.float32)
    for c in range(N_CORES):
        ry, rx = res.results[c]["y"], res.results[c]["x"]
        for b in range(B_LOCAL):
            y_seq[c * B_LOCAL + b] = ry[b].T.astype(np.float32)
            x_seq[c * B_LOCAL + b] = rx[b].T.astype(np.float32)

    # x0 boundary term: x_t += A^t x0, y_t += C A^t x0, t < M_X0.
    At = A.T.astype(np.float64)
    Ct64 = C.T.astype(np.float64)
    xc = x0.astype(np.float64)
    for t in range(M_X0):
        x_seq[:, t, :] += xc.astype(np.float32)
        y_seq[:, t, :] += (xc @ Ct64).astype(np.float32)
        xc = xc @ At
    return (y_seq, x_seq)


# revision 29
# speedup vs baseline: 1.0209x; 1.0193x over previous
"""L2-bounded LTI cell (SSM scan) as a radix-4 tap conv + one doubling
stage on TRN2.

Math: per batch b the reference computes
    x_{t+1} = A x_t + B u_t          (col-vector convention)
    y_t     = C x_t + D u_t
with x_seq[t] = x_t (pre-update), y_seq[t] = y_t, so

    x_t = sum_{m=0}^{t-1} A^m B u_{t-1-m} + A^t x0.

||A^8||_2 ~ 4.5e-2 and the tail decays fast, so truncating at 8 taps
leaves ~2.4e-4 relative error on x (measured in fp64) — far below the
2e-2 gate. The 8-tap causal conv is evaluated per 512-column tile as

    w4 = (B + ABz + A^2Bz^2 + A^3Bz^3) u   -> 4 bf16 matmuls on u
    x  = w4 + A^4 * (w4 shifted by 4)      -> 1 bf16 "doubling" matmul

all accumulated on one PSUM bank (the "+ w4" term is the partial sum
already sitting in PSUM), plus y = D u + C x (bf16 D-term, fp32r
C-term): 7 matmuls/tile vs 28 for the direct tap conv baseline.

Precision (validated in host simulation and on HW): head taps / relay /
D-term in single-pass bf16 contribute errors that are NOT amplified
(they enter through small-gain paths), giving x ~ 3.8e-3. The C-term's
x input is the one place bf16 is not enough (bf16(x) alone costs 4e-2
on y), so x enters y through a rounded fp32r copy and C stays fp32r:
y ~ 1.2e-2 (gate 2e-2). Outputs are stored bf16 (halves output DMA)
and upcast on host.

Engine budget per 512-col tile and core (measured): 7 matmuls ~ 1.9us
on PE (the only engine near saturation); the 3 PSUM->SBUF copies per
batch (w4 bf16, x32 fp32r, y bf16) alternate between DVE and Scalar
(PSUM reads run at ~96 G elem/s, ~0.7us per copy); u loads + y stores
issue on the Sync HWDGE queue, x stores are gpsimd software-DGE
casting DMAs straight from the fp32r x32 tile (SBUF->DRAM bf16),
keeping all DMA off the copy engines.

Sharding: batch 32 -> 4 per core, 8 cores, SPMD, no collectives.
Layout: (d=128 partitions) x (time free dim); host pre-pads/transposes
u to bf16, post-transposes y/x. All 4 batches' buffers are SBUF
resident for the full T=4096 (bf16 makes this fit), and emission
interleaves the 4 batches at matmul granularity so each batch's
matmul -> PSUM-copy -> matmul chain latency hides behind the other
three batches' matmuls. The tiny x0 A^t boundary term is added on host.
"""

from functools import lru_cache

import numpy as np

B_FULL, T, D = 32, 4096, 128
N_CORES = 8
B_LOCAL = B_FULL // N_CORES  # 4

PAD = 8  # left zero-pad of u / w4 (max shift: 4 head taps + relay 4)
M_X0 = 64  # host-side x0-term horizon; ||A^64|| ~ 0
NT = 512  # matmul free dim (one fp32 PSUM bank)
NTILES = T // NT
OUT_CHUNK = 512  # output DMA granularity (cols)

_last_result = None  # BassKernelResults of the most recent run (for test.py)


def _host_matrices(S, K_raw):
    """Mirror reference._ssm_matrices bit-for-bit: fp32 jax on CPU."""
    import jax
    import jax.numpy as jnp

    cpu = jax.devices("cpu")[0]
    with jax.default_device(cpu):
        d_x = S.shape[0]
        sigma = jnp.maximum(jnp.linalg.norm(jnp.asarray(K_raw), ord=2), 1e-5)
        K = jnp.asarray(K_raw) / (sigma + 0.002)
        K11 = K[:d_x, :d_x]
        K12 = K[:d_x, d_x:]
        K21 = K[d_x:, :d_x]
        K22 = K[d_x:, d_x:]
        Sinv = jnp.linalg.inv(jnp.asarray(S))
        A = Sinv @ K11 @ jnp.asarray(S)
        Bm = Sinv @ K12  # GAMMA = 1.0
        C = K21 @ jnp.asarray(S)
        Dm = K22
        return (np.asarray(A), np.asarray(Bm), np.asarray(C), np.asarray(Dm))


@lru_cache(maxsize=2)
def _build():
    import concourse.mybir as mybir
    import concourse.tile as tile
    from concourse import bacc

    F32 = mybir.dt.float32
    F32R = mybir.dt.float32r
    BF16 = mybir.dt.bfloat16
    UW = T + PAD

    nc = bacc.Bacc("TRN2", target_bir_lowering=False, num_devices=N_CORES)
    u_d = nc.dram_tensor("u", [B_LOCAL, D, UW], BF16, kind="ExternalInput")
    gw_d = nc.dram_tensor("gw", [D, 5, D], BF16, kind="ExternalInput")
    cdb_d = nc.dram_tensor("cdb", [D, D], BF16, kind="ExternalInput")
    cdc_d = nc.dram_tensor("cdc", [D, D], F32R, kind="ExternalInput")
    y_d = nc.dram_tensor("y", [B_LOCAL, D, T], BF16, kind="ExternalOutput")
    x_d = nc.dram_tensor("x", [B_LOCAL, D, T], BF16, kind="ExternalOutput")

    with tile.TileContext(nc) as tc:
        with (
            tc.tile_pool(name="const", bufs=1) as const,
            tc.tile_pool(name="upool", bufs=1) as upool,
            tc.tile_pool(name="wpool", bufs=1) as wpool,
            tc.tile_pool(name="x32p", bufs=2) as x32pool,
            tc.tile_pool(name="yacc", bufs=1) as yaccpool,
            tc.tile_pool(name="px", bufs=1, space="PSUM") as px_pool,
            tc.tile_pool(name="py", bufs=1, space="PSUM") as py_pool,
        ):
            gw_sb = const.tile([D, 5, D], BF16)
            nc.sync.dma_start(gw_sb[:], gw_d[:])
            cdb_sb = const.tile([D, D], BF16)
            nc.sync.dma_start(cdb_sb[:], cdb_d[:])
            cdc_sb = const.tile([D, D], F32R)
            nc.sync.dma_start(cdc_sb[:], cdc_d[:])

            u_t, w4, yacc, xtail = [], [], [], []
            for b in range(B_LOCAL):
                ut = upool.tile([D, UW], BF16, name=f"u{b}", tag=f"u{b}")
                u_t.append(ut)
            # u loads: 4 chunks per batch, emitted round-robin across
            # batches so no batch's first tiles wait behind another
            # batch's bulk transfer (the PE queue is in-order).
            bounds = [0, PAD + NT, PAD + NT + 1192, PAD + NT + 2384, UW]
            for ci in range(4):
                lo, hi = bounds[ci], bounds[ci + 1]
                for b in range(B_LOCAL):
                    eng = nc.scalar if (ci == 0 and b % 2) else nc.sync
                    eng.dma_start(u_t[b][:, lo:hi], u_d[b][:, lo:hi])
            for b in range(B_LOCAL):
                wt = wpool.tile([D, UW], BF16, name=f"w{b}", tag=f"w{b}")
                nc.gpsimd.memset(wt[:, :PAD], 0.0)
                w4.append(wt)
                yacc.append(
                    yaccpool.tile([D, T], BF16, name=f"ya{b}", tag=f"ya{b}")
                )
                xtail.append(
                    yaccpool.tile(
                        [D, 4 * NT], BF16, name=f"xt{b}", tag=f"xt{b}"
                    )
                )

            for j in range(NTILES):
                o = j * NT
                px = [None] * B_LOCAL
                py = [None] * B_LOCAL
                x32 = [None] * B_LOCAL
                # head taps: px = sum_m (A^m B) u_{t-1-m}, m = 0..3
                for m in range(4):
                    for b in range(B_LOCAL):
                        if m == 0:
                            px[b] = px_pool.tile(
                                [D, NT], F32, name=f"px{b}", tag=f"px{b}"
                            )
                        nc.tensor.matmul(
                            px[b][:],
                            gw_sb[:, m, :],
                            u_t[b][:, o + PAD - 1 - m : o + PAD - 1 - m + NT],
                            start=(m == 0),
                            stop=False,
                        )
                for b in range(B_LOCAL):
                    eng = nc.vector.tensor_copy if b % 2 == 0 else nc.scalar.copy
                    eng(w4[b][:, o + PAD : o + PAD + NT], px[b][:])
                # relay: px += A^4 * (w4 shifted by 4)  -> x (8 taps)
                for b in range(B_LOCAL):
                    nc.tensor.matmul(
                        px[b][:],
                        gw_sb[:, 4, :],
                        w4[b][:, o + PAD - 4 : o + PAD - 4 + NT],
                        start=False,
                        stop=True,
                    )
                for b in range(B_LOCAL):
                    x32[b] = x32pool.tile(
                        [D, NT], F32R, name=f"x32{b}", tag=f"x32{b}"
                    )
                    eng = nc.scalar.copy if b % 2 == 0 else nc.vector.tensor_copy
                    eng(x32[b][:], px[b][:])
                if j < NTILES - 4:
                    for b in range(B_LOCAL):
                        nc.gpsimd.dma_start(x_d[b][:, o : o + NT], x32[b][:])
                else:
                    ot = (j - (NTILES - 4)) * NT
                    for b in range(B_LOCAL):
                        eng = (
                            nc.vector.tensor_copy
                            if b % 2 == 0
                            else nc.scalar.copy
                        )
                        eng(xtail[b][:, ot : ot + NT], px[b][:])
                    for b in range(B_LOCAL):
                        nc.sync.dma_start(
                            x_d[b][:, o : o + NT], xtail[b][:, ot : ot + NT]
                        )
                # y = D u (bf16) + C x (fp32r)
                for b in range(B_LOCAL):
                    py[b] = py_pool.tile(
                        [D, NT], F32, name=f"py{b}", tag=f"py{b}"
                    )
                    nc.tensor.matmul(
                        py[b][:],
                        cdb_sb[:],
                        u_t[b][:, o + PAD : o + PAD + NT],
                        start=True,
                        stop=False,
                    )
                for b in range(B_LOCAL):
                    nc.tensor.matmul(
                        py[b][:], cdc_sb[:], x32[b][:], start=False, stop=True
                    )
                for b in range(B_LOCAL):
                    eng = nc.scalar.copy if b % 2 == 0 else nc.vector.tensor_copy
                    eng(yacc[b][:, o : o + NT], py[b][:])

                if (o + NT) % OUT_CHUNK == 0:
                    lo = o + NT - OUT_CHUNK
                    for b in range(B_LOCAL):
                        nc.sync.dma_start(
                            y_d[b][:, lo : o + NT], yacc[b][:, lo : o + NT]
                        )
    nc.compile()
    return nc


def _pack_inputs(u, S, K_raw):
    import ml_dtypes

    bf = ml_dtypes.bfloat16
    A, Bm, C, Dm = _host_matrices(S, K_raw)
    A64 = A.astype(np.float64)
    B64 = Bm.astype(np.float64)

    # gw slots 0..3: (A^m B).T head taps; slot 4: (A^4).T relay.
    mats = []
    Am = np.eye(D)
    for m in range(4):
        mats.append((Am @ B64).T)
        Am = A64 @ Am
    mats.append(Am.T)  # (A^4).T relay
    gw_host = np.ascontiguousarray(
        np.stack(mats, axis=1).astype(np.float32)
    ).astype(bf)
    cdb_host = Dm.T.astype(np.float32).astype(bf)
    cdc_host = np.ascontiguousarray(C.T.astype(np.float32))

    in_maps = []
    for c in range(N_CORES):
        up = np.zeros((B_LOCAL, D, PAD + T), dtype=bf)
        for b in range(B_LOCAL):
            up[b, :, PAD:] = u[c * B_LOCAL + b].T.astype(bf)
        in_maps.append(
            {"u": up, "gw": gw_host, "cdb": cdb_host, "cdc": cdc_host}
        )
    return in_maps, A, C


def kernel(u, x0, S, K_raw):
    global _last_result
    from concourse.bass_utils import run_bass_kernel_spmd

    u = np.asarray(u, dtype=np.float32)
    x0 = np.asarray(x0, dtype=np.float32)
    S = np.asarray(S, dtype=np.float32)
    K_raw = np.asarray(K_raw, dtype=np.float32)

    in_maps, A, C = _pack_inputs(u, S, K_raw)
    nc = _build()
    res = run_bass_kernel_spmd(nc, in_maps, core_ids=list(range(N_CORES)))
    _last_result = res

    y_seq = np.empty((B_FULL, T, D), dtype=np.float32)
    x_seq = np.empty((B_FULL, T, D), dtype=np.float32)
    for c in range(N_CORES):
        ry, rx = res.results[c]["y"], res.results[c]["x"]
        for b in range(B_LOCAL):
            y_seq[c * B_LOCAL + b] = ry[b].T.astype(np.float32)
            x_seq[c * B_LOCAL + b] = rx[b].T.astype(np.float32)

    # x0 boundary term: x_t += A^t x0, y_t += C A^t x0, t < M_X0.
    At = A.T.astype(np.float64)
    Ct64 = C.T.astype(np.float64)
    xc = x0.astype(np.float64)
    for t in range(M_X0):
        x_seq[:, t, :] += xc.astype(np.float32)
        y_seq[:, t, :] += (xc @ Ct64).astype(np.float32)
        xc = xc @ At
    return (y_seq, x_seq)


# revision 30
# speedup vs baseline: 1.0278x; 1.0067x over previous
"""L2-bounded LTI cell (SSM scan) as a radix-4 tap conv + one doubling
stage on TRN2.

Math: per batch b the reference computes
    x_{t+1} = A x_t + B u_t          (col-vector convention)
    y_t     = C x_t + D u_t
with x_seq[t] = x_t (pre-update), y_seq[t] = y_t, so

    x_t = sum_{m=0}^{t-1} A^m B u_{t-1-m} + A^t x0.

||A^8||_2 ~ 4.5e-2 and the tail decays fast, so truncating at 8 taps
leaves ~2.4e-4 relative error on x (measured in fp64) — far below the
2e-2 gate. The 8-tap causal conv is evaluated per 512-column tile as

    w4 = (B + ABz + A^2Bz^2 + A^3Bz^3) u   -> 4 bf16 matmuls on u
    x  = w4 + A^4 * (w4 shifted by 4)      -> 1 bf16 "doubling" matmul

all accumulated on one PSUM bank (the "+ w4" term is the partial sum
already sitting in PSUM), plus y = D u + C x (bf16 D-term, fp32r
C-term): 7 matmuls/tile vs 28 for the direct tap conv baseline.

Precision (validated in host simulation and on HW): head taps / relay /
D-term in single-pass bf16 contribute errors that are NOT amplified
(they enter through small-gain paths), giving x ~ 3.8e-3. The C-term's
x input is the one place bf16 is not enough (bf16(x) alone costs 4e-2
on y), so x enters y through a rounded fp32r copy and C stays fp32r:
y ~ 1.2e-2 (gate 2e-2). Outputs are stored bf16 (halves output DMA)
and upcast on host.

Engine budget per 512-col tile and core (measured): 7 matmuls ~ 1.9us
on PE (the only engine near saturation); the 3 PSUM->SBUF copies per
batch (w4 bf16, x32 fp32r, y bf16) alternate between DVE and Scalar
(PSUM reads run at ~96 G elem/s, ~0.7us per copy); u loads + y stores
issue on the Sync HWDGE queue, x stores are gpsimd software-DGE
casting DMAs straight from the fp32r x32 tile (SBUF->DRAM bf16),
keeping all DMA off the copy engines.

Sharding: batch 32 -> 4 per core, 8 cores, SPMD, no collectives.
Layout: (d=128 partitions) x (time free dim); host pre-pads/transposes
u to bf16, post-transposes y/x. All 4 batches' buffers are SBUF
resident for the full T=4096 (bf16 makes this fit), and emission
interleaves the 4 batches at matmul granularity so each batch's
matmul -> PSUM-copy -> matmul chain latency hides behind the other
three batches' matmuls. The tiny x0 A^t boundary term is added on host.
"""

from functools import lru_cache

import numpy as np

B_FULL, T, D = 32, 4096, 128
N_CORES = 8
B_LOCAL = B_FULL // N_CORES  # 4

PAD = 8  # left zero-pad of u / w4 (max shift: 4 head taps + relay 4)
M_X0 = 64  # host-side x0-term horizon; ||A^64|| ~ 0
NT = 512  # matmul free dim (one fp32 PSUM bank)
NTILES = T // NT
OUT_CHUNK = 512  # output DMA granularity (cols)

_last_result = None  # BassKernelResults of the most recent run (for test.py)


def _host_matrices(S, K_raw):
    """Mirror reference._ssm_matrices bit-for-bit: fp32 jax on CPU."""
    import jax
    import jax.numpy as jnp

    cpu = jax.devices("cpu")[0]
    with jax.default_device(cpu):
        d_x = S.shape[0]
        sigma = jnp.maximum(jnp.linalg.norm(jnp.asarray(K_raw), ord=2), 1e-5)
        K = jnp.asarray(K_raw) / (sigma + 0.002)
        K11 = K[:d_x, :d_x]
        K12 = K[:d_x, d_x:]
        K21 = K[d_x:, :d_x]
        K22 = K[d_x:, d_x:]
        Sinv = jnp.linalg.inv(jnp.asarray(S))
        A = Sinv @ K11 @ jnp.asarray(S)
        Bm = Sinv @ K12  # GAMMA = 1.0
        C = K21 @ jnp.asarray(S)
        Dm = K22
        return (np.asarray(A), np.asarray(Bm), np.asarray(C), np.asarray(Dm))


@lru_cache(maxsize=2)
def _build():
    import concourse.mybir as mybir
    import concourse.tile as tile
    from concourse import bacc

    F32 = mybir.dt.float32
    F32R = mybir.dt.float32r
    BF16 = mybir.dt.bfloat16
    UW = T + PAD

    nc = bacc.Bacc("TRN2", target_bir_lowering=False, num_devices=N_CORES)
    u_d = nc.dram_tensor("u", [B_LOCAL, D, UW], BF16, kind="ExternalInput")
    gw_d = nc.dram_tensor("gw", [D, 5, D], BF16, kind="ExternalInput")
    cdb_d = nc.dram_tensor("cdb", [D, D], BF16, kind="ExternalInput")
    cdc_d = nc.dram_tensor("cdc", [D, D], F32R, kind="ExternalInput")
    y_d = nc.dram_tensor("y", [B_LOCAL, D, T], BF16, kind="ExternalOutput")
    x_d = nc.dram_tensor("x", [B_LOCAL, D, T], BF16, kind="ExternalOutput")

    with tile.TileContext(nc) as tc:
        with (
            tc.tile_pool(name="const", bufs=1) as const,
            tc.tile_pool(name="upool", bufs=1) as upool,
            tc.tile_pool(name="wpool", bufs=1) as wpool,
            tc.tile_pool(name="x32p", bufs=2) as x32pool,
            tc.tile_pool(name="yacc", bufs=1) as yaccpool,
            tc.tile_pool(name="px", bufs=1, space="PSUM") as px_pool,
            tc.tile_pool(name="py", bufs=1, space="PSUM") as py_pool,
        ):
            gw_sb = const.tile([D, 5, D], BF16)
            nc.sync.dma_start(gw_sb[:], gw_d[:])
            cdb_sb = const.tile([D, D], BF16)
            nc.sync.dma_start(cdb_sb[:], cdb_d[:])
            cdc_sb = const.tile([D, D], F32R)
            nc.sync.dma_start(cdc_sb[:], cdc_d[:])

            u_t, w4, yacc, xtail = [], [], [], []
            for b in range(B_LOCAL):
                ut = upool.tile([D, UW], BF16, name=f"u{b}", tag=f"u{b}")
                u_t.append(ut)
            # u loads: 4 chunks per batch, emitted round-robin across
            # batches so no batch's first tiles wait behind another
            # batch's bulk transfer (the PE queue is in-order).
            bounds = [0, PAD + NT, PAD + NT + 1192, PAD + NT + 2384, UW]
            for ci in range(4):
                lo, hi = bounds[ci], bounds[ci + 1]
                for b in range(B_LOCAL):
                    eng = nc.scalar if (ci == 0 and b % 2) else nc.sync
                    eng.dma_start(u_t[b][:, lo:hi], u_d[b][:, lo:hi])
            for b in range(B_LOCAL):
                wt = wpool.tile([D, UW], BF16, name=f"w{b}", tag=f"w{b}")
                nc.gpsimd.memset(wt[:, :PAD], 0.0)
                w4.append(wt)
                yacc.append(
                    yaccpool.tile([D, T], BF16, name=f"ya{b}", tag=f"ya{b}")
                )
                xtail.append(
                    yaccpool.tile(
                        [D, 2 * NT], BF16, name=f"xt{b}", tag=f"xt{b}"
                    )
                )

            for j in range(NTILES):
                o = j * NT
                px = [None] * B_LOCAL
                py = [None] * B_LOCAL
                x32 = [None] * B_LOCAL
                # head taps: px = sum_m (A^m B) u_{t-1-m}, m = 0..3
                for m in range(4):
                    for b in range(B_LOCAL):
                        if m == 0:
                            px[b] = px_pool.tile(
                                [D, NT], F32, name=f"px{b}", tag=f"px{b}"
                            )
                        nc.tensor.matmul(
                            px[b][:],
                            gw_sb[:, m, :],
                            u_t[b][:, o + PAD - 1 - m : o + PAD - 1 - m + NT],
                            start=(m == 0),
                            stop=False,
                        )
                for b in range(B_LOCAL):
                    eng = nc.vector.tensor_copy if b % 2 == 0 else nc.scalar.copy
                    eng(w4[b][:, o + PAD : o + PAD + NT], px[b][:])
                # relay: px += A^4 * (w4 shifted by 4)  -> x (8 taps)
                for b in range(B_LOCAL):
                    nc.tensor.matmul(
                        px[b][:],
                        gw_sb[:, 4, :],
                        w4[b][:, o + PAD - 4 : o + PAD - 4 + NT],
                        start=False,
                        stop=True,
                    )
                for b in range(B_LOCAL):
                    x32[b] = x32pool.tile(
                        [D, NT], F32R, name=f"x32{b}", tag=f"x32{b}"
                    )
                    eng = nc.scalar.copy if b % 2 == 0 else nc.vector.tensor_copy
                    eng(x32[b][:], px[b][:])
                if j < NTILES - 2:
                    for b in range(B_LOCAL):
                        nc.gpsimd.dma_start(x_d[b][:, o : o + NT], x32[b][:])
                else:
                    ot = (j - (NTILES - 2)) * NT
                    for b in range(B_LOCAL):
                        eng = (
                            nc.vector.tensor_copy
                            if b % 2 == 0
                            else nc.scalar.copy
                        )
                        eng(xtail[b][:, ot : ot + NT], px[b][:])
                    for b in range(B_LOCAL):
                        nc.sync.dma_start(
                            x_d[b][:, o : o + NT], xtail[b][:, ot : ot + NT]
                        )
                # y = D u (bf16) + C x (fp32r)
                for b in range(B_LOCAL):
                    py[b] = py_pool.tile(
                        [D, NT], F32, name=f"py{b}", tag=f"py{b}"
                    )
                    nc.tensor.matmul(
                        py[b][:],
                        cdb_sb[:],
                        u_t[b][:, o + PAD : o + PAD + NT],
                        start=True,
                        stop=False,
                    )
                for b in range(B_LOCAL):
                    nc.tensor.matmul(
                        py[b][:], cdc_sb[:], x32[b][:], start=False, stop=True
                    )
                for b in range(B_LOCAL):
                    eng = nc.scalar.copy if b % 2 == 0 else nc.vector.tensor_copy
                    eng(yacc[b][:, o : o + NT], py[b][:])

                if (o + NT) % OUT_CHUNK == 0:
                    lo = o + NT - OUT_CHUNK
                    for b in range(B_LOCAL):
                        nc.sync.dma_start(
                            y_d[b][:, lo : o + NT], yacc[b][:, lo : o + NT]
                        )
    nc.compile()
    return nc


def _pack_inputs(u, S, K_raw):
    import ml_dtypes

    bf = ml_dtypes.bfloat16
    A, Bm, C, Dm = _host_matrices(S, K_raw)
    A64 = A.astype(np.float64)
    B64 = Bm.astype(np.float64)

    # gw slots 0..3: (A^m B).T head taps; slot 4: (A^4).T relay.
    mats = []
    Am = np.eye(D)
    for m in range(4):
        mats.append((Am @ B64).T)
        Am = A64 @ Am
    mats.append(Am.T)  # (A^4).T relay
    gw_host = np.ascontiguousarray(
        np.stack(mats, axis=1).astype(np.float32)
    ).astype(bf)
    cdb_host = Dm.T.astype(np.float32).astype(bf)
    cdc_host = np.ascontiguousarray(C.T.astype(np.float32))

    in_maps = []
    for c in range(N_CORES):
        up = np.zeros((B_LOCAL, D, PAD + T), dtype=bf)
        for b in range(B_LOCAL):
            up[b, :, PAD:] = u[c * B_LOCAL + b].T.astype(bf)
        in_maps.append(
            {"u": up, "gw": gw_host, "cdb": cdb_host, "cdc": cdc_host}
        )
    return in_maps, A, C


def kernel(u, x0, S, K_raw):
    global _last_result
    from concourse.bass_utils import run_bass_kernel_spmd

    u = np.asarray(u, dtype=np.float32)
    x0 = np.asarray(x0, dtype=np.float32)
    S = np.asarray(S, dtype=np.float32)
    K_raw = np.asarray(K_raw, dtype=np.float32)

    in_maps, A, C = _pack_inputs(u, S, K_raw)
    nc = _build()
    res = run_bass_kernel_spmd(nc, in_maps, core_ids=list(range(N_CORES)))
    _last_result = res

    y_seq = np.empty((B_FULL, T, D), dtype=np.float32)
    x_seq = np.empty((B_FULL, T, D), dtype=np.float32)
    for c in range(N_CORES):
        ry, rx = res.results[c]["y"], res.results[c]["x"]
        for b in range(B_LOCAL):
            y_seq[c * B_LOCAL + b] = ry[b].T.astype(np.float32)
            x_seq[c * B_LOCAL + b] = rx[b].T.astype(np.float32)

    # x0 boundary term: x_t += A^t x0, y_t += C A^t x0, t < M_X0.
    At = A.T.astype(np.float64)
    Ct64 = C.T.astype(np.float64)
    xc = x0.astype(np.float64)
    for t in range(M_X0):
        x_seq[:, t, :] += xc.astype(np.float32)
        y_seq[:, t, :] += (xc @ Ct64).astype(np.float32)
        xc = xc @ At
    return (y_seq, x_seq)


# revision 31
# speedup vs baseline: 1.1196x; 1.0893x over previous
"""L2-bounded LTI cell (SSM scan) as a radix-4 tap conv + one doubling
stage on TRN2.

Math: per batch b the reference computes
    x_{t+1} = A x_t + B u_t          (col-vector convention)
    y_t     = C x_t + D u_t
with x_seq[t] = x_t (pre-update), y_seq[t] = y_t, so

    x_t = sum_{m=0}^{t-1} A^m B u_{t-1-m} + A^t x0.

||A^8||_2 ~ 4.5e-2 and the tail decays fast, so truncating at 8 taps
leaves ~2.4e-4 relative error on x (measured in fp64) — far below the
2e-2 gate. The 8-tap causal conv is evaluated per 512-column tile as

    w4 = (B + ABz + A^2Bz^2 + A^3Bz^3) u   -> 4 bf16 matmuls on u
    x  = w4 + A^4 * (w4 shifted by 4)      -> 1 bf16 "doubling" matmul

all accumulated on one PSUM bank (the "+ w4" term is the partial sum
already sitting in PSUM), plus y = D u + C x (bf16 D-term, fp32r
C-term): 7 matmuls/tile vs 28 for the direct tap conv baseline.

Precision (validated in host simulation and on HW): head taps / relay /
D-term in single-pass bf16 contribute errors that are NOT amplified
(they enter through small-gain paths), giving x ~ 3.8e-3. The C-term's
x input is the one place bf16 is not enough (bf16(x) alone costs 4e-2
on y), so x enters y through a rounded fp32r copy and C stays fp32r:
y ~ 1.2e-2 (gate 2e-2). Outputs are stored bf16 (halves output DMA)
and upcast on host.

Engine budget per 512-col tile and core (measured): 7 matmuls ~ 1.9us
on PE (the only engine near saturation); the 3 PSUM->SBUF copies per
batch (w4 bf16, x32 fp32r, y bf16) alternate between DVE and Scalar
(PSUM reads run at ~96 G elem/s, ~0.7us per copy); u loads + y stores
issue on the Sync HWDGE queue, x stores are gpsimd software-DGE
casting DMAs straight from the fp32r x32 tile (SBUF->DRAM bf16),
keeping all DMA off the copy engines.

Sharding: batch 32 -> 4 per core, 8 cores, SPMD, no collectives.
Layout: (d=128 partitions) x (time free dim); host pre-pads/transposes
u to bf16, post-transposes y/x. All 4 batches' buffers are SBUF
resident for the full T=4096 (bf16 makes this fit), and emission
interleaves the 4 batches at matmul granularity so each batch's
matmul -> PSUM-copy -> matmul chain latency hides behind the other
three batches' matmuls. The tiny x0 A^t boundary term is added on host.
"""

from functools import lru_cache

import numpy as np

B_FULL, T, D = 32, 4096, 128
N_CORES = 8
B_LOCAL = B_FULL // N_CORES  # 4

PAD = 8  # left zero-pad of u / w4 (max shift: 4 head taps + relay 4)
M_X0 = 64  # host-side x0-term horizon; ||A^64|| ~ 0
NT = 512  # matmul free dim (one fp32 PSUM bank)
NTILES = T // NT
OUT_CHUNK = 512  # output DMA granularity (cols)

_last_result = None  # BassKernelResults of the most recent run (for test.py)


def _host_matrices(S, K_raw):
    """Mirror reference._ssm_matrices bit-for-bit: fp32 jax on CPU."""
    import jax
    import jax.numpy as jnp

    cpu = jax.devices("cpu")[0]
    with jax.default_device(cpu):
        d_x = S.shape[0]
        sigma = jnp.maximum(jnp.linalg.norm(jnp.asarray(K_raw), ord=2), 1e-5)
        K = jnp.asarray(K_raw) / (sigma + 0.002)
        K11 = K[:d_x, :d_x]
        K12 = K[:d_x, d_x:]
        K21 = K[d_x:, :d_x]
        K22 = K[d_x:, d_x:]
        Sinv = jnp.linalg.inv(jnp.asarray(S))
        A = Sinv @ K11 @ jnp.asarray(S)
        Bm = Sinv @ K12  # GAMMA = 1.0
        C = K21 @ jnp.asarray(S)
        Dm = K22
        return (np.asarray(A), np.asarray(Bm), np.asarray(C), np.asarray(Dm))


@lru_cache(maxsize=2)
def _build():
    import concourse.mybir as mybir
    import concourse.tile as tile
    from concourse import bacc

    F32 = mybir.dt.float32
    F32R = mybir.dt.float32r
    BF16 = mybir.dt.bfloat16
    FP16 = mybir.dt.float16
    UW = T + PAD

    nc = bacc.Bacc("TRN2", target_bir_lowering=False, num_devices=N_CORES)
    u_d = nc.dram_tensor("u", [B_LOCAL, D, UW], FP16, kind="ExternalInput")
    gw_d = nc.dram_tensor("gw", [D, 4, D], FP16, kind="ExternalInput")
    cdb_d = nc.dram_tensor("cdb", [D, D], FP16, kind="ExternalInput")
    cdc_d = nc.dram_tensor("cdc", [D, D], FP16, kind="ExternalInput")
    y_d = nc.dram_tensor("y", [B_LOCAL, D, T], BF16, kind="ExternalOutput")
    x_d = nc.dram_tensor("x", [B_LOCAL, D, T], FP16, kind="ExternalOutput")

    with tile.TileContext(nc) as tc:
        with (
            tc.tile_pool(name="const", bufs=1) as const,
            tc.tile_pool(name="upool", bufs=1) as upool,
            tc.tile_pool(name="wpool", bufs=1) as wpool,
            tc.tile_pool(name="x32p", bufs=2) as x32pool,
            tc.tile_pool(name="yacc", bufs=1) as yaccpool,
            tc.tile_pool(name="px", bufs=1, space="PSUM") as px_pool,
            tc.tile_pool(name="py", bufs=1, space="PSUM") as py_pool,
        ):
            gw_sb = const.tile([D, 4, D], FP16)
            nc.sync.dma_start(gw_sb[:], gw_d[:])
            cdb_sb = const.tile([D, D], FP16)
            nc.sync.dma_start(cdb_sb[:], cdb_d[:])
            cdc_sb = const.tile([D, D], FP16)
            nc.sync.dma_start(cdc_sb[:], cdc_d[:])

            u_t, w4, yacc, xtail = [], [], [], []
            for b in range(B_LOCAL):
                ut = upool.tile([D, UW], FP16, name=f"u{b}", tag=f"u{b}")
                u_t.append(ut)
            # u loads: 4 chunks per batch, emitted round-robin across
            # batches so no batch's first tiles wait behind another
            # batch's bulk transfer (the PE queue is in-order).
            bounds = [0, PAD + NT, PAD + NT + 1192, PAD + NT + 2384, UW]
            for ci in range(4):
                lo, hi = bounds[ci], bounds[ci + 1]
                for b in range(B_LOCAL):
                    eng = nc.scalar if (ci == 0 and b % 2) else nc.sync
                    eng.dma_start(u_t[b][:, lo:hi], u_d[b][:, lo:hi])
            for b in range(B_LOCAL):
                wt = wpool.tile([D, UW], FP16, name=f"w{b}", tag=f"w{b}")
                nc.gpsimd.memset(wt[:, :PAD], 0.0)
                w4.append(wt)
                yacc.append(
                    yaccpool.tile([D, T], BF16, name=f"ya{b}", tag=f"ya{b}")
                )
                xtail.append(
                    yaccpool.tile(
                        [D, 2 * NT], FP16, name=f"xt{b}", tag=f"xt{b}"
                    )
                )

            for j in range(NTILES):
                o = j * NT
                px = [None] * B_LOCAL
                py = [None] * B_LOCAL
                x32 = [None] * B_LOCAL
                # head taps: px = sum_m (A^m B) u_{t-1-m}, m = 0..3
                for m in range(3):
                    for b in range(B_LOCAL):
                        if m == 0:
                            px[b] = px_pool.tile(
                                [D, NT], F32, name=f"px{b}", tag=f"px{b}"
                            )
                        nc.tensor.matmul(
                            px[b][:],
                            gw_sb[:, m, :],
                            u_t[b][:, o + PAD - 1 - m : o + PAD - 1 - m + NT],
                            start=(m == 0),
                            stop=False,
                        )
                for b in range(B_LOCAL):
                    eng = nc.vector.tensor_copy if b % 2 == 0 else nc.scalar.copy
                    eng(w4[b][:, o + PAD : o + PAD + NT], px[b][:])
                # relay: px += A^4 * (w4 shifted by 4)  -> x (8 taps)
                for b in range(B_LOCAL):
                    nc.tensor.matmul(
                        px[b][:],
                        gw_sb[:, 3, :],
                        w4[b][:, o + PAD - 3 : o + PAD - 3 + NT],
                        start=False,
                        stop=True,
                    )
                for b in range(B_LOCAL):
                    x32[b] = x32pool.tile(
                        [D, NT], FP16, name=f"x32{b}", tag=f"x32{b}"
                    )
                    eng = nc.scalar.copy if b % 2 == 0 else nc.vector.tensor_copy
                    eng(x32[b][:], px[b][:])
                if j < NTILES - 2:
                    for b in range(B_LOCAL):
                        nc.gpsimd.dma_start(x_d[b][:, o : o + NT], x32[b][:])
                else:
                    ot = (j - (NTILES - 2)) * NT
                    for b in range(B_LOCAL):
                        eng = (
                            nc.vector.tensor_copy
                            if b % 2 == 0
                            else nc.scalar.copy
                        )
                        eng(xtail[b][:, ot : ot + NT], px[b][:])
                    for b in range(B_LOCAL):
                        nc.sync.dma_start(
                            x_d[b][:, o : o + NT], xtail[b][:, ot : ot + NT]
                        )
                # y = D u (bf16) + C x (fp32r)
                for b in range(B_LOCAL):
                    py[b] = py_pool.tile(
                        [D, NT], F32, name=f"py{b}", tag=f"py{b}"
                    )
                    nc.tensor.matmul(
                        py[b][:],
                        cdb_sb[:],
                        u_t[b][:, o + PAD : o + PAD + NT],
                        start=True,
                        stop=False,
                    )
                for b in range(B_LOCAL):
                    nc.tensor.matmul(
                        py[b][:], cdc_sb[:], x32[b][:], start=False, stop=True
                    )
                for b in range(B_LOCAL):
                    eng = nc.scalar.copy if b % 2 == 0 else nc.vector.tensor_copy
                    eng(yacc[b][:, o : o + NT], py[b][:])

                if (o + NT) % OUT_CHUNK == 0:
                    lo = o + NT - OUT_CHUNK
                    for b in range(B_LOCAL):
                        nc.sync.dma_start(
                            y_d[b][:, lo : o + NT], yacc[b][:, lo : o + NT]
                        )
    nc.compile()
    return nc


def _pack_inputs(u, S, K_raw):
    import ml_dtypes

    bf = ml_dtypes.bfloat16
    A, Bm, C, Dm = _host_matrices(S, K_raw)
    A64 = A.astype(np.float64)
    B64 = Bm.astype(np.float64)

    # gw slots 0..2: (A^m B).T head taps; slot 3: (A^3).T relay.
    mats = []
    Am = np.eye(D)
    for m in range(3):
        mats.append((Am @ B64).T)
        Am = A64 @ Am
    mats.append(Am.T)  # (A^3).T relay
    gw_host = np.ascontiguousarray(
        np.stack(mats, axis=1).astype(np.float32)
    ).astype(np.float16)
    cdb_host = Dm.T.astype(np.float16)
    cdc_host = np.ascontiguousarray(C.T.astype(np.float16))

    in_maps = []
    for c in range(N_CORES):
        up = np.zeros((B_LOCAL, D, PAD + T), dtype=np.float16)
        for b in range(B_LOCAL):
            up[b, :, PAD:] = u[c * B_LOCAL + b].T.astype(np.float16)
        in_maps.append(
            {"u": up, "gw": gw_host, "cdb": cdb_host, "cdc": cdc_host}
        )
    return in_maps, A, C


def kernel(u, x0, S, K_raw):
    global _last_result
    from concourse.bass_utils import run_bass_kernel_spmd

    u = np.asarray(u, dtype=np.float32)
    x0 = np.asarray(x0, dtype=np.float32)
    S = np.asarray(S, dtype=np.float32)
    K_raw = np.asarray(K_raw, dtype=np.float32)

    in_maps, A, C = _pack_inputs(u, S, K_raw)
    nc = _build()
    res = run_bass_kernel_spmd(nc, in_maps, core_ids=list(range(N_CORES)))
    _last_result = res

    y_seq = np.empty((B_FULL, T, D), dtype=np.float32)
    x_seq = np.empty((B_FULL, T, D), dtype=np.float32)
    for c in range(N_CORES):
        ry, rx = res.results[c]["y"], res.results[c]["x"]
        for b in range(B_LOCAL):
            y_seq[c * B_LOCAL + b] = ry[b].T.astype(np.float32)
            x_seq[c * B_LOCAL + b] = rx[b].T.astype(np.float32)

    # x0 boundary term: x_t += A^t x0, y_t += C A^t x0, t < M_X0.
    At = A.T.astype(np.float64)
    Ct64 = C.T.astype(np.float64)
    xc = x0.astype(np.float64)
    for t in range(M_X0):
        x_seq[:, t, :] += xc.astype(np.float32)
        y_seq[:, t, :] += (xc @ Ct64).astype(np.float32)
        xc = xc @ At
    return (y_seq, x_seq)
